# revision 26
# baseline (speedup 1.0000x reference)
"""Linformer attention block on 8 TRN2 NeuronCores, data-parallel over batch.

Layout strategy: everything feature-major [C_chunk=128, N] (x arrives [C, N] in
HBM so no input transpose). LayerNorm stats via ones-vector matmuls on the
TensorEngine; K/V produced token-major directly by matmul; dots computed
transposed [k, n] so the softmax denominator is a PE matmul and attn@v needs no
transpose; softmax normalization folded into the attn-output eviction; residual
+ output bias folded into the final PSUM eviction. bf16 matmuls (fp32 PSUM
accumulation), fp32 elementwise / residual path. Heads processed in pairs so
attn tiles die quickly.
"""

import os
import sys
import types

import numpy as np
import ml_dtypes

# The agent image's antenv lacks axon_hooks; bass_utils imports it when
# trace=True under axon. Shim it so tracing degrades gracefully.
try:
    import antenv.axon_hooks  # noqa: F401
except ImportError:
    _shim = types.ModuleType("antenv.axon_hooks")
    _shim.get_axon_ntff_profile_hook = lambda: None
    sys.modules["antenv.axon_hooks"] = _shim

import concourse.bass as bass
import concourse.mybir as mybir
from concourse import bacc
from concourse.tile import TileContext
from concourse.bass_utils import run_bass_kernel_spmd

F32 = mybir.dt.float32
F32R = mybir.dt.float32r
BF16 = mybir.dt.bfloat16
OP = mybir.AluOpType
AF = mybir.ActivationFunctionType

B, C, HH, WW = 32, 512, 32, 32
N = HH * WW            # 1024
HEADS = 8
DH = C // HEADS        # 64
KLR = 256              # linformer rank
EPS = 1e-5
NCORES = 8
BL = B // NCORES       # 4 batch elems per core
CC = C // 128          # 4 channel chunks
NH = N // 512          # 2 free halves
KC = KLR // 128        # 2 k chunks
NT = N // 128          # 8 token chunks


def _rearr(d):
    return d[:].rearrange("(a p) n -> p a n", p=128)


def _build(reps=1):
    nc = bacc.Bacc()
    dp = nc.declare_dram_parameter
    x_d = dp("x", [BL, C, N], F32, isOutput=False)
    posT_d = dp("posT", [C, N], F32, isOutput=False)
    wq_d = dp("wq", [C, C], BF16, isOutput=False)
    wk_d = dp("wk", [C, C], BF16, isOutput=False)
    wv_d = dp("wv", [C, C], BF16, isOutput=False)
    wo_d = dp("wo", [C, C], BF16, isOutput=False)
    pk_d = dp("pk", [N, KLR], BF16, isOutput=False)
    pv_d = dp("pv", [N, KLR], BF16, isOutput=False)
    e2_d = dp("e2", [128, 2, 2], BF16, isOutput=False)
    sel2_d = dp("sel2", [2, 128], BF16, isOutput=False)
    onesb_d = dp("onesb", [128, 1], BF16, isOutput=False)
    onesrow_d = dp("onesrow", [1, 128], BF16, isOutput=False)
    gcol_d = dp("gcol", [128, CC], F32, isOutput=False)
    lnbcol_d = dp("lnbcol", [128, CC], F32, isOutput=False)
    bocol_d = dp("bocol", [128, CC], F32, isOutput=False)
    out_d = dp("out", [BL, C, N], F32, isOutput=True)

    with TileContext(nc) as tc:
        with (
            tc.tile_pool(name="const", bufs=1) as cp,
            tc.tile_pool(name="work", bufs=2) as wp,
            tc.tile_pool(name="ps", bufs=2, space="PSUM") as pp,
        ):
            wq = cp.tile([128, CC, C], BF16)
            nc.sync.dma_start(out=wq, in_=_rearr(wq_d))
            wk = cp.tile([128, CC, C], BF16)
            nc.sync.dma_start(out=wk, in_=_rearr(wk_d))
            wv = cp.tile([128, CC, C], BF16)
            nc.sync.dma_start(out=wv, in_=_rearr(wv_d))
            wo = cp.tile([128, CC, C], BF16)
            nc.sync.dma_start(out=wo, in_=_rearr(wo_d))
            pk = cp.tile([128, NT, KLR], BF16)
            nc.sync.dma_start(out=pk, in_=_rearr(pk_d))
            pv = cp.tile([128, NT, KLR], BF16)
            nc.sync.dma_start(out=pv, in_=_rearr(pv_d))
            e2 = cp.tile([128, 2, 2], BF16)
            nc.sync.dma_start(out=e2, in_=e2_d[:])
            sel2 = cp.tile([2, 128], BF16)
            nc.sync.dma_start(out=sel2, in_=sel2_d[:])
            onesb = cp.tile([128, 1], BF16)
            nc.sync.dma_start(out=onesb, in_=onesb_d[:])
            onesrow = cp.tile([1, 128], BF16)
            nc.sync.dma_start(out=onesrow, in_=onesrow_d[:])
            gcol = cp.tile([128, CC], F32)
            nc.sync.dma_start(out=gcol, in_=gcol_d[:])
            lnbcol = cp.tile([128, CC], F32)
            nc.sync.dma_start(out=lnbcol, in_=lnbcol_d[:])
            bocol = cp.tile([128, CC], F32)
            nc.sync.dma_start(out=bocol, in_=bocol_d[:])
            epsc = cp.tile([1, 1], F32)
            nc.vector.memset(epsc, EPS)

            consts = dict(wq=wq, wk=wk, wv=wv, wo=wo, pk=pk, pv=pv,
                          e2=e2, sel2=sel2, onesb=onesb,
                          onesrow=onesrow, gcol=gcol, lnbcol=lnbcol,
                          bocol=bocol, epsc=epsc)
            with nc.allow_low_precision(reason="bf16 broadcast operands"):
                for _rep in range(reps):
                    for b in range(BL):
                        _emit_batch(nc, wp, pp, b, x_d, posT_d, out_d, consts)
    nc.compile()
    return nc


def _emit_batch(nc, wp, pp, b, x_d, posT_d, out_d, c):
    wq, wk, wv, wo = c["wq"], c["wk"], c["wv"], c["wo"]
    pk, pv, e2, sel2 = c["pk"], c["pv"], c["e2"], c["sel2"]
    onesb, onesrow = c["onesb"], c["onesrow"]
    gcol, lnbcol, bocol, epsc = c["gcol"], c["lnbcol"], c["bocol"], c["epsc"]

    # ---------------- s = x + posT (residual input, fp32) ----------------
    s = wp.tile([128, CC, N], F32, tag="s", bufs=2)
    nc.sync.dma_start(out=s, in_=_rearr(posT_d))
    nc.gpsimd.dma_start(out=s, in_=x_d[b].rearrange("(a p) n -> p a n", p=128),
                        accum_op=OP.add)

    # ---------------- LN statistics (column sums via PE) ----------------
    sqs = []
    sbfs = []
    for cc in range(CC):
        sq = wp.tile([128, N], BF16, tag="sq", bufs=4)
        nc.scalar.activation(sq, s[:, cc, :], AF.Square)
        sqs.append(sq)
        sbf = wp.tile([128, N], BF16, tag="sbf", bufs=4)
        nc.vector.tensor_copy(sbf, s[:, cc, :])
        sbfs.append(sbf)

    mean_bc = wp.tile([128, N], BF16, tag="meanbc", bufs=2)
    rstd_bc = wp.tile([128, N], BF16, tag="rstdbc", bufs=2)
    for nh in range(NH):
        nsl = slice(nh * 512, (nh + 1) * 512)
        s1 = pp.tile([1, 512], F32, tag="st", bufs=2)
        for cc in range(CC):
            nc.tensor.matmul(s1, onesb[:], sbfs[cc][:, nsl],
                             start=(cc == 0), stop=(cc == CC - 1))
        s2 = pp.tile([1, 512], F32, tag="st", bufs=2)
        for cc in range(CC):
            nc.tensor.matmul(s2, onesb[:], sqs[cc][:, nsl],
                             start=(cc == 0), stop=(cc == CC - 1))
        mean = wp.tile([1, 512], BF16, tag="mini", bufs=5)
        nc.vector.tensor_scalar_mul(mean, s1, 1.0 / C)
        m2 = wp.tile([1, 512], F32, tag="mini", bufs=5)
        nc.vector.tensor_mul(m2, mean, s1)  # = C * mean^2
        v512 = wp.tile([1, 512], F32, tag="mini", bufs=5)
        nc.vector.scalar_tensor_tensor(v512, in0=m2, scalar=-1.0, in1=s2,
                                       op0=OP.mult, op1=OP.add)  # C*var
        sd = wp.tile([1, 512], F32, tag="mini", bufs=5)
        nc.scalar.activation(sd, v512, AF.Sqrt, bias=epsc[:], scale=1.0 / C)
        rstd = wp.tile([1, 512], BF16, tag="mini", bufs=5)
        nc.vector.reciprocal(rstd, sd)
        # broadcast mean/rstd down 128 partitions: PE outer product + evict
        mb_ps = pp.tile([128, 512], F32, tag="mm", bufs=6)
        nc.tensor.matmul(mb_ps, onesrow[:], mean[:], start=True, stop=True)
        nc.scalar.copy(mean_bc[:, nsl], mb_ps)
        rb_ps = pp.tile([128, 512], F32, tag="mm", bufs=6)
        nc.tensor.matmul(rb_ps, onesrow[:], rstd[:], start=True, stop=True)
        nc.scalar.copy(rstd_bc[:, nsl], rb_ps)

    # ---------------- LN apply + relu -> ybf (bf16) ----------------
    ybf = wp.tile([128, CC, N], BF16, tag="ybf", bufs=2)
    for cc in range(CC):
        t = wp.tile([128, N], F32, tag="lnt", bufs=2)
        nc.vector.scalar_tensor_tensor(t, in0=s[:, cc, :], scalar=0.0,
                                       in1=mean_bc, op0=OP.bypass,
                                       op1=OP.subtract)
        nc.vector.scalar_tensor_tensor(t, in0=t, scalar=gcol[:, cc:cc + 1],
                                       in1=rstd_bc, op0=OP.mult, op1=OP.mult)
        nc.scalar.activation(ybf[:, cc, :], t, AF.Relu,
                             bias=lnbcol[:, cc:cc + 1])

    # ---------------- projections: q^T, K_tok, V_tok ----------------
    qbf = wp.tile([128, CC, N], BF16, tag="qbf", bufs=2)   # [d_part, dc, n]
    for dc in range(CC):
        for nh in range(NH):
            nsl = slice(nh * 512, (nh + 1) * 512)
            ps = pp.tile([128, 512], F32, tag="mm", bufs=6)
            for kc in range(CC):
                nc.tensor.matmul(ps, wq[:, kc, dc * 128:(dc + 1) * 128],
                                 ybf[:, kc, nsl],
                                 start=(kc == 0), stop=(kc == CC - 1))
            nc.vector.tensor_copy(qbf[:, dc, nsl], ps)

    kbf = wp.tile([128, NT, C], BF16, tag="kbf", bufs=2)   # [n_part, nt, d]
    for t in range(NT):
        ps = pp.tile([128, 512], F32, tag="mm", bufs=6)
        for kc in range(CC):
            nc.tensor.matmul(ps, ybf[:, kc, t * 128:(t + 1) * 128],
                             wk[:, kc, :], start=(kc == 0), stop=(kc == CC - 1))
        nc.scalar.copy(kbf[:, t, :], ps)

    vbf = wp.tile([128, NT, C], BF16, tag="vbf", bufs=2)   # [n_part, nt, d]
    for t in range(NT):
        ps = pp.tile([128, 512], F32, tag="mm", bufs=6)
        for kc in range(CC):
            nc.tensor.matmul(ps, ybf[:, kc, t * 128:(t + 1) * 128],
                             wv[:, kc, :], start=(kc == 0), stop=(kc == CC - 1))
        nc.vector.tensor_copy(vbf[:, t, :], ps)

    # ---------------- k_^T [d_part, dc, K] and v_ [k_part, kc, d] ----------
    ktbf = wp.tile([128, CC, KLR], BF16, tag="ktbf", bufs=1)
    for dc in range(CC):
        ps = pp.tile([128, KLR], F32, tag="mm", bufs=6)
        for t in range(NT):
            nc.tensor.matmul(ps, kbf[:, t, dc * 128:(dc + 1) * 128],
                             pk[:, t, :], start=(t == 0), stop=(t == NT - 1))
        nc.scalar.copy(ktbf[:, dc, :], ps)

    vsbf = wp.tile([128, KC, C], BF16, tag="vsbf", bufs=1)
    for kc in range(KC):
        ps = pp.tile([128, 512], F32, tag="mm", bufs=6)
        for t in range(NT):
            nc.tensor.matmul(ps, pv[:, t, kc * 128:(kc + 1) * 128],
                             vbf[:, t, :], start=(t == 0), stop=(t == NT - 1))
        nc.vector.tensor_copy(vsbf[:, kc, :], ps)

    # ---------------- attention, one head-pair at a time ----------------
    aobf = wp.tile([128, CC, N], BF16, tag="aobf", bufs=2)
    for pr in range(CC):  # head pair (2pr, 2pr+1) == cin chunk pr
        attn = {}
        sums = []
        for nh in range(NH):
            sums.append(pp.tile([2, 512], F32, tag="st", bufs=2,
                                name=f"sums_{b}_{pr}_{nh}"))
        for kc in range(KC):
            ksl = slice(kc * 128, (kc + 1) * 128)
            for hp in range(2):
                at = wp.tile([128, N], BF16, tag="attn", bufs=4,
                             name=f"at_{b}_{pr}_{kc}_{hp}")
                attn[(hp, kc)] = at
            for nh in range(NH):
                nsl = slice(nh * 512, (nh + 1) * 512)
                dps = {}
                # both heads' dots issued back-to-back into disjoint PE
                # row groups (contraction rows 0:64 / 64:128) -> concurrent
                for hp, r in ((0, 0), (1, 64)):
                    rsl = slice(r, r + 64)
                    d = pp.tile([128, 512], F32, tag="mm", bufs=6,
                                name=f"dps_{b}_{pr}_{kc}_{nh}_{hp}")
                    dps[hp] = d
                    nc.tensor.matmul(d, ktbf[rsl, pr, ksl],
                                     qbf[rsl, pr, nsl], start=True, stop=True)
                for hp in range(2):
                    nc.scalar.activation(attn[(hp, kc)][:, nsl], dps[hp],
                                         AF.Exp)
                    nc.tensor.matmul(sums[nh], e2[:, hp, :],
                                     attn[(hp, kc)][:, nsl],
                                     start=(kc == 0 and hp == 0),
                                     stop=(kc == KC - 1 and hp == 1),
                                     skip_group_check=True)
        recip = wp.tile([2, N], BF16, tag="recip", bufs=2)
        rbc = wp.tile([128, N], BF16, tag="rbc", bufs=2)
        for nh in range(NH):
            nsl = slice(nh * 512, (nh + 1) * 512)
            nc.vector.reciprocal(recip[:, nsl], sums[nh])
            rb_ps = pp.tile([128, 512], F32, tag="mm", bufs=6)
            nc.tensor.matmul(rb_ps, sel2[:], recip[:, nsl],
                             start=True, stop=True)
            nc.scalar.copy(rbc[:, nsl], rb_ps)
        for nh in range(NH):
            nsl = slice(nh * 512, (nh + 1) * 512)
            aps = pp.tile([128, 512], F32, tag="mm", bufs=6)
            for r, hp in ((0, 0), (64, 1)):
                h = 2 * pr + hp
                for kc in range(KC):
                    nc.tensor.matmul(aps[r:r + 64, :],
                                     vsbf[:, kc, h * 64:(h + 1) * 64],
                                     attn[(hp, kc)][:, nsl],
                                     start=(kc == 0),
                                     stop=(kc == KC - 1),
                                     tile_position=(0, 64) if r else None,
                                     skip_group_check=True)
            nc.vector.scalar_tensor_tensor(aobf[:, pr, nsl], in0=aps,
                                           scalar=0.0, in1=rbc[:, nsl],
                                           op0=OP.bypass, op1=OP.mult)

    # ---------------- Wo + bias + residual -> out ----------------
    for co in range(CC):
        outf = wp.tile([128, N], F32, tag="outf", bufs=2)
        for nh in range(NH):
            nsl = slice(nh * 512, (nh + 1) * 512)
            ps = pp.tile([128, 512], F32, tag="mm", bufs=6)
            for kc in range(CC):
                nc.tensor.matmul(ps, wo[:, kc, co * 128:(co + 1) * 128],
                                 aobf[:, kc, nsl],
                                 start=(kc == 0), stop=(kc == CC - 1))
            nc.vector.scalar_tensor_tensor(outf[:, nsl], in0=ps,
                                           scalar=bocol[:, co:co + 1],
                                           in1=s[:, co, nsl],
                                           op0=OP.add, op1=OP.add)
        nc.sync.dma_start(out=out_d[b, co * 128:(co + 1) * 128, :], in_=outf)


_CACHE = {}


def get_nc(reps=1):
    key = ("nc", reps)
    if key not in _CACHE:
        _CACHE[key] = _build(reps)
    return _CACHE[key]


def make_in_maps(inputs):
    bf = ml_dtypes.bfloat16
    x = np.ascontiguousarray(np.asarray(inputs["x"], np.float32)
                             .reshape(B, C, N))
    pos = np.asarray(inputs["pos"], np.float32).reshape(N, C)
    ln_g = np.asarray(inputs["ln_g"], np.float32)
    ln_b = np.asarray(inputs["ln_b"], np.float32)
    bo = np.asarray(inputs["bo"], np.float32)

    e2 = np.zeros((128, 2, 2), bf)
    e2[:, 0, 0] = 1
    e2[:, 1, 1] = 1
    sel2 = np.zeros((2, 128), bf)
    sel2[0, :64] = 1
    sel2[1, 64:] = 1

    shared = {
        "posT": np.ascontiguousarray(pos.T),
        "wq": (np.asarray(inputs["Wq"], np.float32) * DH ** -0.5).astype(bf),
        "wk": np.asarray(inputs["Wk"], np.float32).astype(bf),
        "wv": np.asarray(inputs["Wv"], np.float32).astype(bf),
        "wo": np.asarray(inputs["Wo"], np.float32).astype(bf),
        "pk": np.asarray(inputs["proj_k"], np.float32).astype(bf),
        "pv": np.asarray(inputs["proj_v"], np.float32).astype(bf),
        "e2": e2,
        "sel2": sel2,
        "onesb": np.ones((128, 1), bf),
        "onesrow": np.ones((1, 128), bf),
        "gcol": np.ascontiguousarray(ln_g.reshape(CC, 128).T),
        "lnbcol": np.ascontiguousarray(ln_b.reshape(CC, 128).T),
        "bocol": np.ascontiguousarray(bo.reshape(CC, 128).T),
    }
    return [dict(shared, x=np.ascontiguousarray(x[i * BL:(i + 1) * BL]))
            for i in range(NCORES)]


def kernel(**inputs):
    nc = get_nc()
    in_maps = make_in_maps(inputs)
    trace = bool(int(os.environ.get("BASS_KERNEL_TRACE", "0")))
    res = run_bass_kernel_spmd(nc, in_maps, core_ids=list(range(NCORES)),
                               trace=trace)
    kernel.last_result = res
    out = np.concatenate([np.asarray(res.results[i]["out"], np.float32)
                          [None] for i in range(NCORES)], axis=0)
    return np.ascontiguousarray(out.reshape(B, C, HH, WW))


# revision 28
# speedup vs baseline: 1.5434x; 1.5434x over previous
"""Linformer attention block on 8 TRN2 NeuronCores, data-parallel over batch.

Layout strategy: everything feature-major [C_chunk=128, N] (x arrives [C, N] in
HBM so no input transpose). LayerNorm stats via ones-vector matmuls on the
TensorEngine; K/V produced token-major directly by matmul; dots computed
transposed [k, n] so the softmax denominator is a PE matmul and attn@v needs no
transpose; softmax normalization folded into the attn-output eviction; residual
+ output bias folded into the final PSUM eviction. bf16 matmuls (fp32 PSUM
accumulation), fp32 elementwise / residual path. Heads processed in pairs so
attn tiles die quickly.
"""

import os
import sys
import types

import numpy as np
import ml_dtypes

# The agent image's antenv lacks axon_hooks; bass_utils imports it when
# trace=True under axon. Shim it so tracing degrades gracefully.
try:
    import antenv.axon_hooks  # noqa: F401
except ImportError:
    _shim = types.ModuleType("antenv.axon_hooks")
    _shim.get_axon_ntff_profile_hook = lambda: None
    sys.modules["antenv.axon_hooks"] = _shim

import concourse.bass as bass
import concourse.mybir as mybir
from concourse import bacc
from concourse.tile import TileContext
from concourse.bass_utils import run_bass_kernel_spmd

F32 = mybir.dt.float32
F32R = mybir.dt.float32r
BF16 = mybir.dt.bfloat16
OP = mybir.AluOpType
AF = mybir.ActivationFunctionType

B, C, HH, WW = 32, 512, 32, 32
N = HH * WW            # 1024
HEADS = 8
DH = C // HEADS        # 64
KLR = 256              # linformer rank
EPS = 1e-5
NCORES = 8
BL = B // NCORES       # 4 batch elems per core
CC = C // 128          # 4 channel chunks
NH = N // 512          # 2 free halves
KC = KLR // 128        # 2 k chunks
NT = N // 128          # 8 token chunks


def _rearr(d):
    return d[:].rearrange("(a p) n -> p a n", p=128)


def _build(reps=1):
    nc = bacc.Bacc()
    dp = nc.declare_dram_parameter
    x_d = dp("x", [BL, C, N], F32, isOutput=False)
    posT_d = dp("posT", [C, N], F32, isOutput=False)
    wq_d = dp("wq", [C, C], BF16, isOutput=False)
    wk_d = dp("wk", [C, C], BF16, isOutput=False)
    wv_d = dp("wv", [C, C], BF16, isOutput=False)
    wo_d = dp("wo", [C, C], BF16, isOutput=False)
    pk_d = dp("pk", [N, KLR], BF16, isOutput=False)
    pv_d = dp("pv", [N, KLR], BF16, isOutput=False)
    e2_d = dp("e2", [128, 2, 2], BF16, isOutput=False)
    sel2_d = dp("sel2", [2, 128], BF16, isOutput=False)
    onesb_d = dp("onesb", [128, 1], BF16, isOutput=False)
    onesrow_d = dp("onesrow", [1, 128], BF16, isOutput=False)
    gcol_d = dp("gcol", [128, CC], F32, isOutput=False)
    lnbcol_d = dp("lnbcol", [128, CC], F32, isOutput=False)
    bocol_d = dp("bocol", [128, CC], F32, isOutput=False)
    out_d = dp("out", [BL, C, N], F32, isOutput=True)

    with TileContext(nc) as tc:
        with (
            tc.tile_pool(name="const", bufs=1) as cp,
            tc.tile_pool(name="work", bufs=2) as wp,
            tc.tile_pool(name="ps", bufs=2, space="PSUM") as pp,
        ):
            wq = cp.tile([128, CC, C], BF16)
            nc.sync.dma_start(out=wq, in_=_rearr(wq_d))
            wk = cp.tile([128, CC, C], BF16)
            nc.sync.dma_start(out=wk, in_=_rearr(wk_d))
            wv = cp.tile([128, CC, C], BF16)
            nc.sync.dma_start(out=wv, in_=_rearr(wv_d))
            wo = cp.tile([128, CC, C], BF16)
            nc.sync.dma_start(out=wo, in_=_rearr(wo_d))
            pk = cp.tile([128, NT, KLR], BF16)
            nc.sync.dma_start(out=pk, in_=_rearr(pk_d))
            pv = cp.tile([128, NT, KLR], BF16)
            nc.sync.dma_start(out=pv, in_=_rearr(pv_d))
            e2 = cp.tile([128, 2, 2], BF16)
            nc.sync.dma_start(out=e2, in_=e2_d[:])
            sel2 = cp.tile([2, 128], BF16)
            nc.sync.dma_start(out=sel2, in_=sel2_d[:])
            onesb = cp.tile([128, 1], BF16)
            nc.sync.dma_start(out=onesb, in_=onesb_d[:])
            onesrow = cp.tile([1, 128], BF16)
            nc.sync.dma_start(out=onesrow, in_=onesrow_d[:])
            gcol = cp.tile([128, CC], F32)
            nc.sync.dma_start(out=gcol, in_=gcol_d[:])
            lnbcol = cp.tile([128, CC], F32)
            nc.sync.dma_start(out=lnbcol, in_=lnbcol_d[:])
            bocol = cp.tile([128, CC], F32)
            nc.sync.dma_start(out=bocol, in_=bocol_d[:])
            epsc = cp.tile([1, 1], F32)
            nc.vector.memset(epsc, EPS)

            consts = dict(wq=wq, wk=wk, wv=wv, wo=wo, pk=pk, pv=pv,
                          e2=e2, sel2=sel2, onesb=onesb,
                          onesrow=onesrow, gcol=gcol, lnbcol=lnbcol,
                          bocol=bocol, epsc=epsc)
            with nc.allow_low_precision(reason="bf16 broadcast operands"):
                for _rep in range(reps):
                    for b in range(BL):
                        _emit_batch(nc, wp, pp, b, x_d, posT_d, out_d, consts)
    nc.compile()
    return nc


def _emit_batch(nc, wp, pp, b, x_d, posT_d, out_d, c):
    front = _emit_front(nc, wp, pp, b, x_d, posT_d, c)
    _emit_back(nc, wp, pp, b, out_d, c, front)


def _emit_front(nc, wp, pp, b, x_d, posT_d, c):
    wq, wk, wv, wo = c["wq"], c["wk"], c["wv"], c["wo"]
    pk, pv, e2, sel2 = c["pk"], c["pv"], c["e2"], c["sel2"]
    onesb, onesrow = c["onesb"], c["onesrow"]
    gcol, lnbcol, bocol, epsc = c["gcol"], c["lnbcol"], c["bocol"], c["epsc"]

    # ---------------- s = x + posT (residual input, fp32) ----------------
    s = wp.tile([128, CC, N], F32, tag="s", bufs=2)
    nc.sync.dma_start(out=s, in_=_rearr(posT_d))
    nc.gpsimd.dma_start(out=s, in_=x_d[b].rearrange("(a p) n -> p a n", p=128),
                        accum_op=OP.add)

    # ---------------- LN statistics (column sums via PE) ----------------
    sqs = []
    sbfs = []
    for cc in range(CC):
        sq = wp.tile([128, N], BF16, tag="sq", bufs=4)
        nc.scalar.activation(sq, s[:, cc, :], AF.Square)
        sqs.append(sq)
        sbf = wp.tile([128, N], BF16, tag="sbf", bufs=4)
        nc.vector.tensor_copy(sbf, s[:, cc, :])
        sbfs.append(sbf)

    mean_bc = wp.tile([128, N], BF16, tag="meanbc", bufs=2)
    rstd_bc = wp.tile([128, N], BF16, tag="rstdbc", bufs=2)
    for nh in range(NH):
        nsl = slice(nh * 512, (nh + 1) * 512)
        s1 = pp.tile([1, 512], F32, tag="st", bufs=2)
        for cc in range(CC):
            nc.tensor.matmul(s1, onesb[:], sbfs[cc][:, nsl],
                             start=(cc == 0), stop=(cc == CC - 1))
        s2 = pp.tile([1, 512], F32, tag="st", bufs=2)
        for cc in range(CC):
            nc.tensor.matmul(s2, onesb[:], sqs[cc][:, nsl],
                             start=(cc == 0), stop=(cc == CC - 1))
        mean = wp.tile([1, 512], BF16, tag="mini", bufs=5)
        nc.vector.tensor_scalar_mul(mean, s1, 1.0 / C)
        m2 = wp.tile([1, 512], F32, tag="mini", bufs=5)
        nc.vector.tensor_mul(m2, mean, s1)  # = C * mean^2
        v512 = wp.tile([1, 512], F32, tag="mini", bufs=5)
        nc.vector.scalar_tensor_tensor(v512, in0=m2, scalar=-1.0, in1=s2,
                                       op0=OP.mult, op1=OP.add)  # C*var
        sd = wp.tile([1, 512], F32, tag="mini", bufs=5)
        nc.scalar.activation(sd, v512, AF.Sqrt, bias=epsc[:], scale=1.0 / C)
        rstd = wp.tile([1, 512], BF16, tag="mini", bufs=5)
        nc.vector.reciprocal(rstd, sd)
        # broadcast mean/rstd down 128 partitions: PE outer product + evict
        mb_ps = pp.tile([128, 512], F32, tag="mm", bufs=6)
        nc.tensor.matmul(mb_ps, onesrow[:], mean[:], start=True, stop=True)
        nc.scalar.copy(mean_bc[:, nsl], mb_ps)
        rb_ps = pp.tile([128, 512], F32, tag="mm", bufs=6)
        nc.tensor.matmul(rb_ps, onesrow[:], rstd[:], start=True, stop=True)
        nc.scalar.copy(rstd_bc[:, nsl], rb_ps)

    # ---------------- LN apply + relu -> ybf (bf16) ----------------
    ybf = wp.tile([128, CC, N], BF16, tag="ybf", bufs=2)
    for cc in range(CC):
        t = wp.tile([128, N], F32, tag="lnt", bufs=2)
        nc.vector.scalar_tensor_tensor(t, in0=s[:, cc, :], scalar=0.0,
                                       in1=mean_bc, op0=OP.bypass,
                                       op1=OP.subtract)
        nc.vector.scalar_tensor_tensor(t, in0=t, scalar=gcol[:, cc:cc + 1],
                                       in1=rstd_bc, op0=OP.mult, op1=OP.mult)
        nc.scalar.activation(ybf[:, cc, :], t, AF.Relu,
                             bias=lnbcol[:, cc:cc + 1])

    # ---------------- projections: q^T, K_tok, V_tok ----------------
    qbf = wp.tile([128, CC, N], BF16, tag="qbf", bufs=2)   # [d_part, dc, n]
    for dc in range(CC):
        for nh in range(NH):
            nsl = slice(nh * 512, (nh + 1) * 512)
            ps = pp.tile([128, 512], F32, tag="mm", bufs=6)
            for kc in range(CC):
                nc.tensor.matmul(ps, wq[:, kc, dc * 128:(dc + 1) * 128],
                                 ybf[:, kc, nsl],
                                 start=(kc == 0), stop=(kc == CC - 1))
            nc.vector.tensor_copy(qbf[:, dc, nsl], ps)

    kbf = wp.tile([128, NT, C], BF16, tag="kbf", bufs=2)   # [n_part, nt, d]
    for t in range(NT):
        ps = pp.tile([128, 512], F32, tag="mm", bufs=6)
        for kc in range(CC):
            nc.tensor.matmul(ps, ybf[:, kc, t * 128:(t + 1) * 128],
                             wk[:, kc, :], start=(kc == 0), stop=(kc == CC - 1))
        nc.scalar.copy(kbf[:, t, :], ps)

    vbf = wp.tile([128, NT, C], BF16, tag="vbf", bufs=2)   # [n_part, nt, d]
    for t in range(NT):
        ps = pp.tile([128, 512], F32, tag="mm", bufs=6)
        for kc in range(CC):
            nc.tensor.matmul(ps, ybf[:, kc, t * 128:(t + 1) * 128],
                             wv[:, kc, :], start=(kc == 0), stop=(kc == CC - 1))
        nc.vector.tensor_copy(vbf[:, t, :], ps)

    # ---------------- k_^T [d_part, dc, K] and v_ [k_part, kc, d] ----------
    ktbf = wp.tile([128, CC, KLR], BF16, tag="ktbf", bufs=1)
    for dc in range(CC):
        ps = pp.tile([128, KLR], F32, tag="mm", bufs=6)
        for t in range(NT):
            nc.tensor.matmul(ps, kbf[:, t, dc * 128:(dc + 1) * 128],
                             pk[:, t, :], start=(t == 0), stop=(t == NT - 1))
        nc.scalar.copy(ktbf[:, dc, :], ps)

    vsbf = wp.tile([128, KC, C], BF16, tag="vsbf", bufs=1)
    for kc in range(KC):
        ps = pp.tile([128, 512], F32, tag="mm", bufs=6)
        for t in range(NT):
            nc.tensor.matmul(ps, pv[:, t, kc * 128:(kc + 1) * 128],
                             vbf[:, t, :], start=(t == 0), stop=(t == NT - 1))
        nc.vector.tensor_copy(vsbf[:, kc, :], ps)

    return dict(s=s, qbf=qbf, ktbf=ktbf, vsbf=vsbf)


def _emit_back(nc, wp, pp, b, out_d, c, front):
    wq, wk, wv, wo = c["wq"], c["wk"], c["wv"], c["wo"]
    pk, pv, e2, sel2 = c["pk"], c["pv"], c["e2"], c["sel2"]
    onesb, onesrow = c["onesb"], c["onesrow"]
    gcol, lnbcol, bocol, epsc = c["gcol"], c["lnbcol"], c["bocol"], c["epsc"]
    s, qbf, ktbf, vsbf = (front["s"], front["qbf"], front["ktbf"],
                          front["vsbf"])

    # ---------------- attention, one head-pair at a time ----------------
    aobf = wp.tile([128, CC, N], BF16, tag="aobf", bufs=2)
    for pr in range(CC):  # head pair (2pr, 2pr+1) == cin chunk pr
        attn = {}
        sums = []
        for nh in range(NH):
            sums.append(pp.tile([2, 512], F32, tag="st", bufs=2,
                                name=f"sums_{b}_{pr}_{nh}"))
        for kc in range(KC):
            ksl = slice(kc * 128, (kc + 1) * 128)
            for hp in range(2):
                at = wp.tile([128, N], BF16, tag="attn", bufs=4,
                             name=f"at_{b}_{pr}_{kc}_{hp}")
                attn[(hp, kc)] = at
            for nh in range(NH):
                nsl = slice(nh * 512, (nh + 1) * 512)
                dps = {}
                # both heads' dots issued back-to-back into disjoint PE
                # row groups (contraction rows 0:64 / 64:128) -> concurrent
                for hp, r in ((0, 0), (1, 64)):
                    rsl = slice(r, r + 64)
                    d = pp.tile([128, 512], F32, tag="mm", bufs=6,
                                name=f"dps_{b}_{pr}_{kc}_{nh}_{hp}")
                    dps[hp] = d
                    nc.tensor.matmul(d, ktbf[rsl, pr, ksl],
                                     qbf[rsl, pr, nsl], start=True, stop=True)
                for hp in range(2):
                    nc.scalar.activation(attn[(hp, kc)][:, nsl], dps[hp],
                                         AF.Exp)
                    nc.tensor.matmul(sums[nh], e2[:, hp, :],
                                     attn[(hp, kc)][:, nsl],
                                     start=(kc == 0 and hp == 0),
                                     stop=(kc == KC - 1 and hp == 1),
                                     skip_group_check=True)
        recip = wp.tile([2, N], BF16, tag="recip", bufs=2)
        rbc = wp.tile([128, N], BF16, tag="rbc", bufs=2)
        for nh in range(NH):
            nsl = slice(nh * 512, (nh + 1) * 512)
            nc.vector.reciprocal(recip[:, nsl], sums[nh])
            rb_ps = pp.tile([128, 512], F32, tag="mm", bufs=6)
            nc.tensor.matmul(rb_ps, sel2[:], recip[:, nsl],
                             start=True, stop=True)
            nc.scalar.copy(rbc[:, nsl], rb_ps)
        for nh in range(NH):
            nsl = slice(nh * 512, (nh + 1) * 512)
            aps = pp.tile([128, 512], F32, tag="mm", bufs=6)
            for r, hp in ((0, 0), (64, 1)):
                h = 2 * pr + hp
                for kc in range(KC):
                    nc.tensor.matmul(aps[r:r + 64, :],
                                     vsbf[:, kc, h * 64:(h + 1) * 64],
                                     attn[(hp, kc)][:, nsl],
                                     start=(kc == 0),
                                     stop=(kc == KC - 1),
                                     tile_position=(0, 64) if r else None,
                                     skip_group_check=True)
            nc.vector.scalar_tensor_tensor(aobf[:, pr, nsl], in0=aps,
                                           scalar=0.0, in1=rbc[:, nsl],
                                           op0=OP.bypass, op1=OP.mult)

    # ---------------- Wo + bias + residual -> out ----------------
    for co in range(CC):
        outf = wp.tile([128, N], F32, tag="outf", bufs=2)
        for nh in range(NH):
            nsl = slice(nh * 512, (nh + 1) * 512)
            ps = pp.tile([128, 512], F32, tag="mm", bufs=6)
            for kc in range(CC):
                nc.tensor.matmul(ps, wo[:, kc, co * 128:(co + 1) * 128],
                                 aobf[:, kc, nsl],
                                 start=(kc == 0), stop=(kc == CC - 1))
            nc.vector.scalar_tensor_tensor(outf[:, nsl], in0=ps,
                                           scalar=bocol[:, co:co + 1],
                                           in1=s[:, co, nsl],
                                           op0=OP.add, op1=OP.add)
        nc.sync.dma_start(out=out_d[b, co * 128:(co + 1) * 128, :], in_=outf)


_CACHE = {}


def get_nc(reps=1):
    key = ("nc", reps)
    if key not in _CACHE:
        _CACHE[key] = _build(reps)
    return _CACHE[key]


def make_in_maps(inputs):
    bf = ml_dtypes.bfloat16
    x = np.ascontiguousarray(np.asarray(inputs["x"], np.float32)
                             .reshape(B, C, N))
    pos = np.asarray(inputs["pos"], np.float32).reshape(N, C)
    ln_g = np.asarray(inputs["ln_g"], np.float32)
    ln_b = np.asarray(inputs["ln_b"], np.float32)
    bo = np.asarray(inputs["bo"], np.float32)

    e2 = np.zeros((128, 2, 2), bf)
    e2[:, 0, 0] = 1
    e2[:, 1, 1] = 1
    sel2 = np.zeros((2, 128), bf)
    sel2[0, :64] = 1
    sel2[1, 64:] = 1

    shared = {
        "posT": np.ascontiguousarray(pos.T),
        "wq": (np.asarray(inputs["Wq"], np.float32) * DH ** -0.5).astype(bf),
        "wk": np.asarray(inputs["Wk"], np.float32).astype(bf),
        "wv": np.asarray(inputs["Wv"], np.float32).astype(bf),
        "wo": np.asarray(inputs["Wo"], np.float32).astype(bf),
        "pk": np.asarray(inputs["proj_k"], np.float32).astype(bf),
        "pv": np.asarray(inputs["proj_v"], np.float32).astype(bf),
        "e2": e2,
        "sel2": sel2,
        "onesb": np.ones((128, 1), bf),
        "onesrow": np.ones((1, 128), bf),
        "gcol": np.ascontiguousarray(ln_g.reshape(CC, 128).T),
        "lnbcol": np.ascontiguousarray(ln_b.reshape(CC, 128).T),
        "bocol": np.ascontiguousarray(bo.reshape(CC, 128).T),
    }
    return [dict(shared, x=np.ascontiguousarray(x[i * BL:(i + 1) * BL]))
            for i in range(NCORES)]


def kernel(**inputs):
    nc = get_nc()
    in_maps = make_in_maps(inputs)
    trace = bool(int(os.environ.get("BASS_KERNEL_TRACE", "0")))
    res = run_bass_kernel_spmd(nc, in_maps, core_ids=list(range(NCORES)),
                               trace=trace)
    kernel.last_result = res
    out = np.concatenate([np.asarray(res.results[i]["out"], np.float32)
                          [None] for i in range(NCORES)], axis=0)
    return np.ascontiguousarray(out.reshape(B, C, HH, WW))
